# revision 1
# baseline (speedup 1.0000x reference)
"""DiffS6 (differential Mamba selective-scan block) TRN2 Bass kernel.

Strategy: d_inner is sharded 8 ways (256 channels/core). Each core:
  in_proj (PE, fp16) -> causal conv + silu -> x_proj partials (PE)
  -> AllReduce(dt/B/C projections, 1.5MB) -> dt_proj + softplus (PE/ACT)
  -> per (branch, n): dA = exp(A_n * delta) on ACT, dBu on DVE/GPSIMD,
     h = tensor_tensor_scan on DVE (the only engine with the scan op),
     y += C_n * h (DVE tree) -> gate with silu(z), D-term -> out_proj (PE).
Each core emits an fp16 [1024, 2048] partial of out^T; host sums and
transposes. All 16-bit tensors are fp16 (bf16's 8-bit mantissa breaks the
decay factors dA: 5.6e-2 rel err vs 7e-4 for fp16).
"""
import numpy as np

NCORES = 8
D_MODEL = 1024
D_INNER = 2048
D_STATE = 16
D_CONV = 4
DT_RANK = 64
L = 2048
DLOC = D_INNER // NCORES      # 256
NDT = DLOC // 128             # 2 d-tiles per core
P = 128
TC = 512                      # matmul free-dim chunk
NTC = L // TC                 # 4
NKC = D_MODEL // P            # 8

_CACHE = {}


def _build():
    import concourse.mybir as mybir
    import concourse.tile as tile
    from concourse import bacc

    F32 = mybir.dt.float32
    F16 = mybir.dt.float16
    AT = mybir.ActivationFunctionType
    OP = mybir.AluOpType

    nc = bacc.Bacc("TRN2", target_bir_lowering=False, debug=False,
                   enable_asserts=False, num_devices=NCORES)

    # ---- per-core external inputs ----
    hT_d = nc.dram_tensor("hT", [D_MODEL, L], F16, kind="ExternalInput")
    ipwT_d = nc.dram_tensor("ipwT", [D_MODEL, 2 * DLOC], F16, kind="ExternalInput")
    convw_d = nc.dram_tensor("convw", [DLOC, D_CONV], F32, kind="ExternalInput")
    convb_d = nc.dram_tensor("convb", [DLOC, 1], F32, kind="ExternalInput")
    xpwT_d = nc.dram_tensor("xpwT", [DLOC, 192], F16, kind="ExternalInput")
    dtpwT_d = nc.dram_tensor("dtpwT", [2, DT_RANK, DLOC], F32, kind="ExternalInput")
    dtb_d = nc.dram_tensor("dtb", [2, DLOC, 1], F32, kind="ExternalInput")
    acol_d = nc.dram_tensor("acol", [2, DLOC, D_STATE], F32, kind="ExternalInput")
    ddiff_d = nc.dram_tensor("ddiff", [DLOC, 1], F32, kind="ExternalInput")
    opwT_d = nc.dram_tensor("opwT", [DLOC, D_MODEL], F16, kind="ExternalInput")
    out_d = nc.dram_tensor("outp", [NDT, D_MODEL, L], F16, kind="ExternalOutput")

    # collective bounce buffers (DRAM)
    dbldt_in = nc.dram_tensor("dbldt_in", [2, 2, DT_RANK, L // 2], F32, kind="Internal")
    dbldt_out = nc.dram_tensor("dbldt_out", [2, 2, DT_RANK, L // 2], F32,
                               kind="Internal", addr_space="Shared")
    dblbc_in = nc.dram_tensor("dblbc_in", [2, 32, L], F16, kind="Internal")
    dblbc_out = nc.dram_tensor("dblbc_out", [2, 32, L], F16,
                               kind="Internal", addr_space="Shared")

    with tile.TileContext(nc) as tc:
        with tc.tile_pool(name="wts", bufs=1) as wp, \
             tc.tile_pool(name="ht", bufs=3) as hp, \
             tc.tile_pool(name="big", bufs=1) as bigp, \
             tc.tile_pool(name="stage", bufs=1) as stp, \
             tc.tile_pool(name="dv", bufs=4) as dvp, \
             tc.tile_pool(name="bc", bufs=2) as bcp, \
             tc.tile_pool(name="da", bufs=3) as dap, \
             tc.tile_pool(name="h", bufs=3) as hpool, \
             tc.tile_pool(name="g", bufs=6) as gp, \
             tc.tile_pool(name="conv", bufs=2) as cvp, \
             tc.tile_pool(name="y", bufs=1) as yp, \
             tc.tile_pool(name="ya", bufs=4) as yap, \
             tc.tile_pool(name="osb", bufs=1) as op_, \
             tc.tile_pool(name="mm", bufs=4, space="PSUM") as mmp, \
             tc.tile_pool(name="mm2", bufs=2, space="PSUM") as mmp2:

            # ---- load weights ----
            ipwT = []
            for kc in range(NKC):
                t = wp.tile([P, 2 * DLOC], F16, tag=f"ipwT{kc}")
                nc.sync.dma_start(t[:], ipwT_d[kc * P:(kc + 1) * P, :])
                ipwT.append(t)
            xpwT = []
            for dt in range(NDT):
                t = wp.tile([P, 192], F16, tag=f"xpwT{dt}")
                nc.sync.dma_start(t[:], xpwT_d[dt * P:(dt + 1) * P, :])
                xpwT.append(t)
            dtpwT = []
            for br in range(2):
                t = wp.tile([DT_RANK, DLOC], F32, tag=f"dtpwT{br}")
                nc.sync.dma_start(t[:], dtpwT_d[br])
                dtpwT.append(t)
            opwT = []
            for dt in range(NDT):
                t = wp.tile([P, D_MODEL], F16, tag=f"opwT{dt}")
                nc.sync.dma_start(t[:], opwT_d[dt * P:(dt + 1) * P, :])
                opwT.append(t)
            convw, convb, ddiff = [], [], []
            dtb, acol = {}, {}
            for dt in range(NDT):
                t = wp.tile([P, D_CONV], F32, tag=f"convw{dt}")
                nc.sync.dma_start(t[:], convw_d[dt * P:(dt + 1) * P, :])
                convw.append(t)
                t = wp.tile([P, 1], F32, tag=f"convb{dt}")
                nc.sync.dma_start(t[:], convb_d[dt * P:(dt + 1) * P, :])
                convb.append(t)
                t = wp.tile([P, 1], F32, tag=f"ddiff{dt}")
                nc.sync.dma_start(t[:], ddiff_d[dt * P:(dt + 1) * P, :])
                ddiff.append(t)
                for br in range(2):
                    t = wp.tile([P, 1], F32, tag=f"dtb{br}{dt}")
                    nc.sync.dma_start(t[:], dtb_d[br, dt * P:(dt + 1) * P, :])
                    dtb[br, dt] = t
                    t = wp.tile([P, D_STATE], F32, tag=f"acol{br}{dt}")
                    nc.sync.dma_start(t[:], acol_d[br, dt * P:(dt + 1) * P, :])
                    acol[br, dt] = t

            # ---- persistent activations ----
            # x padded with 3 leading zeros for the causal conv
            x16 = [bigp.tile([P, L + 3], F16, tag=f"x16_{dt}", name=f"x16_{dt}") for dt in range(NDT)]
            z16 = [bigp.tile([P, L], F16, tag=f"z16_{dt}", name=f"z16_{dt}") for dt in range(NDT)]
            u16 = [bigp.tile([P, L], F16, tag=f"u16_{dt}", name=f"u16_{dt}") for dt in range(NDT)]
            for dt in range(NDT):
                nc.vector.memset(x16[dt][:, 0:3], 0.0)

            # ---- P1a: in_proj x-rows only (z deferred past the collective) ----
            for tcc in range(NTC):
                pss = [mmp.tile([P, TC], F32, tag="mm", name=f"ps{i}")
                       for i in range(2)]
                for kc in range(NKC):
                    ht = hp.tile([P, TC], F16, tag="ht")
                    nc.sync.dma_start(ht[:], hT_d[kc * P:(kc + 1) * P,
                                                  tcc * TC:(tcc + 1) * TC])
                    for rt in range(2):
                        nc.tensor.matmul(pss[rt][:],
                                         ipwT[kc][:, rt * P:(rt + 1) * P],
                                         ht[:], start=(kc == 0),
                                         stop=(kc == NKC - 1))
                for rt in range(2):
                    nc.scalar.copy(x16[rt][:, 3 + tcc * TC:3 + (tcc + 1) * TC],
                                   pss[rt][:])

                # conv + silu + x_proj for this chunk
                for dt in range(NDT):
                    cacc = cvp.tile([P, TC], F16, tag="conv")
                    s, e = tcc * TC, (tcc + 1) * TC
                    nc.vector.tensor_scalar(cacc[:], x16[dt][:, s:e],
                                            convw[dt][:, 0:1], None, OP.mult)
                    for k in range(1, D_CONV):
                        cacc2 = cvp.tile([P, TC], F16, tag="conv")
                        nc.vector.scalar_tensor_tensor(
                            cacc2[:], x16[dt][:, s + k:e + k],
                            convw[dt][:, k:k + 1],
                            cacc[:], OP.mult, OP.add)
                        cacc = cacc2
                    nc.scalar.activation(u16[dt][:, s:e], cacc[:], AT.Silu,
                                         bias=convb[dt][:, 0:1], scale=1.0)
                for br in range(2):
                    ps = mmp2.tile([96, TC], F32, tag="mm96")
                    for dt in range(NDT):
                        nc.tensor.matmul(ps[:], xpwT[dt][:, br * 96:(br + 1) * 96],
                                         u16[dt][:, tcc * TC:(tcc + 1) * TC],
                                         start=(dt == 0), stop=(dt == NDT - 1))
                    evd = cvp.tile([DT_RANK, TC], F32, tag="dbl_ev", name="evd")
                    nc.scalar.copy(evd[:], ps[0:DT_RANK, :])
                    half, off = tcc // 2, (tcc % 2) * TC
                    nc.sync.dma_start(dbldt_in[br, half, :, off:off + TC],
                                      evd[:])
                    evb = cvp.tile([32, TC], F16, tag="ev_bc", name="evb")
                    nc.scalar.copy(evb[:], ps[DT_RANK:96, :])
                    nc.sync.dma_start(dblbc_in[br, :, tcc * TC:(tcc + 1) * TC],
                                      evb[:])

            # three AllReduces: br0 dt-rows fp32, all B/C rows fp16, br1
            # dt-rows fp32. B/C output feeds the broadcast DMAs directly.
            nc.gpsimd.collective_compute(
                "AllReduce", OP.add,
                replica_groups=[list(range(NCORES))],
                ins=[dbldt_in[0].opt()],
                outs=[dbldt_out[0].opt()],
            )
            nc.gpsimd.collective_compute(
                "AllReduce", OP.add,
                replica_groups=[list(range(NCORES))],
                ins=[dblbc_in[:].opt()],
                outs=[dblbc_out[:].opt()],
            )
            nc.gpsimd.collective_compute(
                "AllReduce", OP.add,
                replica_groups=[list(range(NCORES))],
                ins=[dbldt_in[1].opt()],
                outs=[dbldt_out[1].opt()],
            )

            # load reduced dt_raw rows fp32
            dtr = []
            for br in range(2):
                t = stp.tile([DT_RANK, L], F32, tag=f"dtr{br}")
                for half in range(2):
                    hs = half * (L // 2)
                    nc.sync.dma_start(t[:, hs:hs + L // 2], dbldt_out[br, half])
                dtr.append(t)

            # ---- P5/P6: software-pipelined scan phase ----
            # iteration order: (dt0,br0), (dt1,br0), (dt0,br1), (dt1,br1).
            # delta/v prep for an iteration is emitted ahead of time so the
            # ACT exp/ln burst never starves the DVE of dA tiles.
            iters = [(0, 0), (1, 0), (0, 1), (1, 1)]
            prepped = {}

            def prep(k):
                dt, br = iters[k]
                delta = dvp.tile([P, L], F16, tag="delta", name=f"delta{k}")
                for tcc in range(NTC):
                    ps = mmp2.tile([P, TC], F32, tag="mmdt", name="psd")
                    nc.tensor.matmul(ps[:], dtpwT[br][:, dt * P:(dt + 1) * P],
                                     dtr[br][:, tcc * TC:(tcc + 1) * TC],
                                     start=True, stop=True)
                    # softplus(x) = ln(exp(x) + 1); x observed in [-9, 0]
                    nc.scalar.activation(delta[:, tcc * TC:(tcc + 1) * TC],
                                         ps[:], AT.Exp,
                                         bias=dtb[br, dt][:, 0:1], scale=1.0)
                    nc.scalar.activation(delta[:, tcc * TC:(tcc + 1) * TC],
                                         delta[:, tcc * TC:(tcc + 1) * TC],
                                         AT.Ln, bias=1.0)
                v16 = dvp.tile([P, L], F16, tag="v16", name=f"v16_{k}")
                nc.vector.tensor_tensor(v16[:], delta[:], u16[dt][:], OP.mult)
                prepped[k] = (delta, v16)

            prep(0)
            prep(1)

            # ---- P1b: in_proj z-rows (needed only for the final gating) ----
            for tcc in range(NTC):
                pss = [mmp.tile([P, TC], F32, tag="mm", name=f"psz{i}")
                       for i in range(2)]
                for kc in range(NKC):
                    ht = hp.tile([P, TC], F16, tag="ht")
                    nc.sync.dma_start(ht[:], hT_d[kc * P:(kc + 1) * P,
                                                  tcc * TC:(tcc + 1) * TC])
                    for rt in range(2):
                        nc.tensor.matmul(pss[rt][:],
                                         ipwT[kc][:, (rt + 2) * P:(rt + 3) * P],
                                         ht[:], start=(kc == 0),
                                         stop=(kc == NKC - 1))
                for rt in range(2):
                    nc.scalar.copy(z16[rt][:, tcc * TC:(tcc + 1) * TC],
                                   pss[rt][:])


            y0 = {}      # branch-0 scan output per d-tile
            pending_out = []   # (dt, ygt) whose out_proj is interleaved later

            def emit_out(dt, ygt, ot):
                osb = op_.tile([P, L], F16, tag="osb", name="osb")
                for tcc in range(NTC):
                    ps = mmp.tile([P, TC], F32, tag="mm", name="pso")
                    nc.tensor.matmul(ps[:], opwT[dt][:, ot * P:(ot + 1) * P],
                                     ygt[:, tcc * TC:(tcc + 1) * TC],
                                     start=True, stop=True)
                    nc.scalar.copy(osb[:, tcc * TC:(tcc + 1) * TC], ps[:])
                nc.sync.dma_start(out_d[dt, ot * P:(ot + 1) * P, :], osb[:])

            for k, (dt, br) in enumerate(iters):
                delta, v16 = prepped.pop(k)
                # y accumulation over n: ping-pong mult-then-add
                ytot = yap.tile([P, L], F16, tag="ya", name="ytot")
                for n in range(D_STATE):
                    if k + 2 < len(iters) and n == 3:
                        prep(k + 2)
                    if pending_out and 6 <= n <= 13:
                        emit_out(pending_out[0][0], pending_out[0][1], n - 6)
                        if n == 13:
                            pending_out.pop(0)
                    bb = bcp.tile([P, L], F16, tag="bb")
                    nc.sync.dma_start(
                        bb[:], dblbc_out[br, n:n + 1, :].broadcast_to((P, L)))
                    cb = bcp.tile([P, L], F16, tag="cb")
                    nc.sync.dma_start(
                        cb[:], dblbc_out[br, D_STATE + n:D_STATE + n + 1,
                                         :].broadcast_to((P, L)))
                    dA = dap.tile([P, L], F16, tag="dA")
                    nc.scalar.activation(dA[:], delta[:], AT.Exp, bias=0.0,
                                         scale=acol[br, dt][:, n:n + 1])
                    dBu = dap.tile([P, L], F16, tag="dBu")
                    nc.vector.tensor_tensor(dBu[:], v16[:], bb[:], OP.mult)
                    hh = hpool.tile([P, L], F16, tag="h")
                    nc.vector.tensor_tensor_scan(hh[:], dA[:], dBu[:], 0.0,
                                                 OP.mult, OP.add)
                    if n == 0:
                        nc.vector.tensor_tensor(ytot[:], hh[:], cb[:], OP.mult)
                    else:
                        g = gp.tile([P, L], F16, tag="g")
                        nc.vector.tensor_tensor(g[:], hh[:], cb[:], OP.mult)
                        if n == D_STATE - 1 and br == 0:
                            ynew = yp.tile([P, L], F16, tag=f"y0_{dt}",
                                           name=f"y0_{dt}")
                        else:
                            ynew = yap.tile([P, L], F16, tag="ya", name="ynew")
                        nc.vector.tensor_tensor(ynew[:], ytot[:], g[:], OP.add)
                        ytot = ynew
                total = ytot

                if br == 0:
                    y0[dt] = total
                    continue

                # ---- P7 + P8 for this d-tile (overlaps the next iteration) --
                yd = gp.tile([P, L], F16, tag="g")
                nc.vector.tensor_tensor(yd[:], y0[dt][:], total[:], OP.subtract)
                yd2 = gp.tile([P, L], F16, tag="g")
                nc.vector.scalar_tensor_tensor(
                    yd2[:], u16[dt][:], ddiff[dt][:, 0:1], yd[:],
                    OP.mult, OP.add)
                sz = gp.tile([P, L], F16, tag="g")
                nc.scalar.activation(sz[:], z16[dt][:], AT.Silu)
                ygt = bigp.tile([P, L + 3], F16, tag=f"x16_{dt}")
                nc.vector.tensor_tensor(ygt[:, 0:L], yd2[:], sz[:], OP.mult)
                if k == len(iters) - 1:
                    for ot in range(D_MODEL // P):
                        emit_out(dt, ygt, ot)
                else:
                    pending_out.append((dt, ygt))

    nc.finalize()
    return nc


def _get_nc():
    if "nc" not in _CACHE:
        _CACHE["nc"] = _build()
    return _CACHE["nc"]


def kernel(hidden_states, in_proj_w, conv_w, conv_b,
           x1_proj_w, dt1_proj_w, dt1_proj_b, A1_log, D1,
           x2_proj_w, dt2_proj_w, dt2_proj_b, A2_log, D2,
           out_proj_w):
    import os
    from concourse.bass_utils import run_bass_kernel_spmd
    try:
        import antenv.axon_hooks  # noqa: F401
    except ImportError:
        # tracing needs the axon NTFF hook; without it a stray BASS_TRACE
        # env var would crash run_bass_kernel_spmd
        os.environ["BASS_NEVER_TRACE"] = "1"

    f32 = np.float32
    f16 = np.float16
    hidden_states = np.asarray(hidden_states, f32)
    in_proj_w = np.asarray(in_proj_w, f32)
    conv_w = np.asarray(conv_w, f32)
    conv_b = np.asarray(conv_b, f32)
    out_proj_w = np.asarray(out_proj_w, f32)

    hT16 = np.ascontiguousarray(hidden_states[0].T).astype(f16)  # (1024, 2048)
    A1 = -np.exp(np.asarray(A1_log, f32))
    A2 = -np.exp(np.asarray(A2_log, f32))
    Dd = (np.asarray(D1, f32) - np.asarray(D2, f32))

    xp = [np.asarray(x1_proj_w, f32), np.asarray(x2_proj_w, f32)]
    dtpw = [np.asarray(dt1_proj_w, f32), np.asarray(dt2_proj_w, f32)]
    dtb = [np.asarray(dt1_proj_b, f32), np.asarray(dt2_proj_b, f32)]
    Acols = [A1, A2]

    in_maps = []
    for c in range(NCORES):
        ds = slice(c * DLOC, (c + 1) * DLOC)
        ipw_loc = np.concatenate([in_proj_w[ds], in_proj_w[D_INNER:][ds]], 0)
        in_maps.append({
            "hT": hT16,
            "ipwT": np.ascontiguousarray(ipw_loc.T).astype(f16),
            "convw": np.ascontiguousarray(conv_w[ds]).astype(f32),
            "convb": np.ascontiguousarray(conv_b[ds][:, None]).astype(f32),
            "xpwT": np.ascontiguousarray(
                np.concatenate([xp[0][:, ds], xp[1][:, ds]], 0).T).astype(f16),
            "dtpwT": np.ascontiguousarray(
                np.stack([dtpw[0][ds].T, dtpw[1][ds].T])).astype(f32),
            "dtb": np.ascontiguousarray(
                np.stack([dtb[0][ds][:, None], dtb[1][ds][:, None]])).astype(f32),
            "acol": np.ascontiguousarray(
                np.stack([Acols[0][ds], Acols[1][ds]])).astype(f32),
            "ddiff": np.ascontiguousarray(Dd[ds][:, None]).astype(f32),
            "opwT": np.ascontiguousarray(out_proj_w[:, ds].T).astype(f16),
        })

    nc = _get_nc()
    res = run_bass_kernel_spmd(nc, in_maps, core_ids=list(range(NCORES)))
    _CACHE["last_res"] = res
    out = np.zeros((D_MODEL, L), f32)
    for r in res.results:
        out += r["outp"].astype(f32).sum(axis=0)
    return np.ascontiguousarray(out.T)[None].astype(f32)

